# revision 22
# baseline (speedup 1.0000x reference)
"""MoE gate (64 experts, top-6) on 8 Trainium2 NeuronCores, data-parallel.

Full inputs in, full outputs out. Token dim (N=262144) is sharded across the
8 cores; the tiny (64,128) gate weight is replicated. Per core the Bass/Tile
kernel computes, for each 128-token tile:
  PE   : logits = x_tile @ W.T            (fp32 matmul, x-slice stationary)
  ACT  : scores = exp(logits), Z = row-sum (fused accum_out)
  DVE  : top-8 score values + indices (max / max_index), 1/Z (batched recip)
  PE   : Pi-partial += rZ^T @ scores      (float32r stats matmul into PSUM)
Top-8 values/indices are staged in SBUF with a host-chosen token permutation
so every DMA is large and contiguous. The host finishes the job: normalizes
top-6 weights, bincounts expert usage, averages the Pi partials and forms the
aux loss. A small host-side repair pass recomputes tokens whose top-8 scores
have near-tie adjacent gaps (the ACT exp is a ~1e-5 approximation, so ties
within that noise could otherwise be ordered differently than the f32
reference).
"""

import os

import numpy as np

N_CORES = 8
HID = 128
NE = 64  # experts
TOPK = 6
ALPHA = 1e-3
N_TOK = 64 * 4096  # 262144 tokens total
TPC = N_TOK // N_CORES  # 32768 tokens per core
TILE = 128  # tokens per compute tile
NTILES = TPC // TILE  # 256
GROUP = 8  # tiles per group (ACT/stats batching)
CHUNK_TOK = 8192  # tokens per input DMA chunk
NCHUNK = TPC // CHUNK_TOK  # 4
TILES_PER_CHUNK = CHUNK_TOK // TILE  # 64
GROUPS_PER_CHUNK = TILES_PER_CHUNK // GROUP  # 8
NGROUPS = NCHUNK * GROUPS_PER_CHUNK  # 32

# relative gap under which adjacent top-8 ranks are considered ambiguous and
# the token is recomputed exactly on the host
TIE_REL_THRESH = 3e-4

_CACHE = {}


def _build():
    import concourse.bacc as bacc
    import concourse.mybir as mybir
    import concourse.tile as tile

    nc = bacc.Bacc("TRN2", target_bir_lowering=False, debug=False,
                   num_devices=N_CORES)
    f32 = mybir.dt.float32
    bf16 = mybir.dt.bfloat16
    u32 = mybir.dt.uint32
    Exp = mybir.ActivationFunctionType.Exp

    # x and W are shipped as bf16 hi/lo pairs; logits are rebuilt as
    # xh*wh + xh*wl + xl*wh + xl*wl (error ~1e-5, fixed by host repair)
    xh = nc.dram_tensor("xh", [HID, TPC], bf16, kind="ExternalInput")
    xl = nc.dram_tensor("xl", [HID, TPC], bf16, kind="ExternalInput")
    wt = nc.dram_tensor("wt", [HID, 2 * NE], bf16, kind="ExternalInput")
    # row 0: iota64 tiled (index-stuff pattern), row 1: 0xFFFFFFC0 mask
    consts = nc.dram_tensor("consts", [2, GROUP * NE], u32, kind="ExternalInput")
    # vals DRAM layout [128, NTILES*8]: partition p, free i*8+k holds the
    # rank-k stuffed score of local token p*NTILES+i (host permutes inputs
    # to match; expert index lives in the low 6 mantissa bits).
    vals = nc.dram_tensor("vals", [TILE, NTILES * 8], f32, kind="ExternalOutput")
    pi = nc.dram_tensor("pi", [GROUP, GROUP * NE], f32, kind="ExternalOutput")

    with tile.TileContext(nc) as tc:
        with (
            tc.tile_pool(name="singles", bufs=1) as singles,
            tc.tile_pool(name="xchunks", bufs=2) as xpool,
            tc.tile_pool(name="scorep", bufs=4) as spool,
            tc.tile_pool(name="zp", bufs=4) as zpool,
            tc.tile_pool(name="stagep", bufs=2) as stpool,
            tc.tile_pool(name="psuml", bufs=2, space="PSUM") as lpool,
            tc.tile_pool(name="psumstat", bufs=1, space="PSUM") as statpool,
        ):
            w_sb = singles.tile([HID, 2 * NE], bf16)
            nc.sync.dma_start(out=w_sb, in_=wt[:, :])
            iota_sb = singles.tile([TILE, GROUP * NE], u32)
            nc.sync.dma_start(
                out=iota_sb,
                in_=consts[0:1, :].to_broadcast([TILE, GROUP * NE]))
            mask_sb = singles.tile([TILE, 1], u32)
            nc.sync.dma_start(
                out=mask_sb, in_=consts[1:2, 0:1].to_broadcast([TILE, 1]))
            stats = statpool.tile([GROUP, GROUP * NE], f32)
            pending_stats = []

            for c in range(NCHUNK):
                xh_t = xpool.tile([HID, CHUNK_TOK], bf16)
                nc.sync.dma_start(
                    out=xh_t, in_=xh[:, c * CHUNK_TOK:(c + 1) * CHUNK_TOK])
                xl_t = xpool.tile([HID, CHUNK_TOK], bf16)
                nc.sync.dma_start(
                    out=xl_t, in_=xl[:, c * CHUNK_TOK:(c + 1) * CHUNK_TOK])
                stage_v = stpool.tile([TILE, TILES_PER_CHUNK * 8], f32)
                for g in range(GROUPS_PER_CHUNK):
                    gi = c * GROUPS_PER_CHUNK + g
                    if g % 2 == 0:
                        zpair = zpool.tile([TILE, 2 * GROUP], bf16)
                        rzpair = zpool.tile([TILE, 2 * GROUP], bf16)
                    lg = lpool.tile([TILE, GROUP * NE], f32)
                    scores = spool.tile([TILE, GROUP * NE], f32)
                    stuffed = spool.tile([TILE, GROUP * NE], u32)
                    scores_bf = spool.tile([TILE, GROUP * NE], bf16)
                    zblk = zpair[:, (g % 2) * GROUP:(g % 2) * GROUP + GROUP]
                    rz = rzpair[:, (g % 2) * GROUP:(g % 2) * GROUP + GROUP]
                    for m in range(GROUP):
                        t = g * GROUP + m  # tile within chunk
                        xs = slice(t * TILE, (t + 1) * TILE)
                        out = lg[:, m * NE:(m + 1) * NE]
                        nc.tensor.matmul(out, xh_t[:, xs], w_sb[:, :NE],
                                         start=True, stop=False)
                        nc.tensor.matmul(out, xh_t[:, xs], w_sb[:, NE:],
                                         start=False, stop=False)
                        nc.tensor.matmul(out, xl_t[:, xs], w_sb[:, :NE],
                                         start=False, stop=True)
                    nc.scalar.activation(scores[:, :], lg[:, :], Exp)
                    # overwrite each score's low 6 mantissa bits with its
                    # expert index: order-preserving, host decodes bits&63
                    nc.vector.scalar_tensor_tensor(
                        stuffed[:, :],
                        scores[:, :].bitcast(u32),
                        mask_sb[:, :],
                        iota_sb[:, :],
                        op0=mybir.AluOpType.bitwise_and,
                        op1=mybir.AluOpType.bitwise_or,
                    )
                    stuffed_f = stuffed[:, :].bitcast(f32)
                    with nc.allow_low_precision(
                            reason="Z/rz feed Pi stats only; bf16 noise "
                                   "averages out over 262k tokens"):
                        nc.scalar.copy(scores_bf[:, :], scores[:, :])
                        nc.vector.tensor_reduce(
                            zblk,
                            scores_bf[:, :].rearrange("p (g e) -> p g e",
                                                      e=NE),
                            axis=mybir.AxisListType.X,
                            op=mybir.AluOpType.add,
                        )
                        pending_stats.append((rz, scores_bf, gi))
                        if g % 2 == 1:
                            nc.vector.reciprocal(rzpair[:, :], zpair[:, :])
                            for prz, pbf, pgi in pending_stats:
                                nc.tensor.matmul(
                                    stats[:, :],
                                    prz,
                                    pbf[:, :],
                                    start=(pgi == 0),
                                    stop=(pgi == NGROUPS - 1),
                                )
                            pending_stats.clear()
                    for m in range(GROUP):
                        t = g * GROUP + m
                        nc.vector.max(
                            stage_v[:, t * 8:t * 8 + 8],
                            stuffed[:, m * NE:(m + 1) * NE].bitcast(f32),
                        )
                s0 = c * TILES_PER_CHUNK * 8
                s1 = (c + 1) * TILES_PER_CHUNK * 8
                nc.scalar.dma_start(out=vals[:, s0:s1], in_=stage_v)

            stats_sb = singles.tile([GROUP, GROUP * NE], f32)
            nc.vector.tensor_copy(stats_sb, stats)
            nc.scalar.dma_start(out=pi[:, :], in_=stats_sb)

    nc.compile()
    return nc


def _get_nc():
    if "nc" not in _CACHE:
        _CACHE["nc"] = _build()
    return _CACHE["nc"]


def _ensure_axon_ntff_hook():
    """Provide antenv.axon_hooks if the image lacks it (NTFF tracing)."""
    import contextlib
    import ctypes
    import sys
    import types

    try:
        from antenv.axon_hooks import get_axon_ntff_profile_hook  # noqa: F401
        return
    except ImportError:
        pass

    so_path = "/opt/axon/libaxon_pjrt.so"
    if not os.path.exists(so_path):
        return
    lib = ctypes.CDLL(so_path)
    if not hasattr(lib, "axon_start_nrt_profile"):
        return
    lib.axon_start_nrt_profile.argtypes = [
        ctypes.POINTER(ctypes.c_int64), ctypes.c_size_t]
    lib.axon_start_nrt_profile.restype = ctypes.c_int64
    lib.axon_stop_nrt_profile.argtypes = [ctypes.c_char_p]
    lib.axon_stop_nrt_profile.restype = ctypes.c_int64

    @contextlib.contextmanager
    def _hook(output_dir, device_ids):
        import jax
        jax.devices()
        if device_ids:
            ids = (ctypes.c_int64 * len(device_ids))(*device_ids)
            rc = lib.axon_start_nrt_profile(ids, len(device_ids))
        else:
            rc = lib.axon_start_nrt_profile(None, 0)
        if rc != 0:
            raise RuntimeError(f"axon_start_nrt_profile rc={rc}")
        try:
            yield
        finally:
            n = lib.axon_stop_nrt_profile(str(output_dir).encode())
            print(f"profile: {n} file(s) written to {output_dir}",
                  file=sys.stderr)

    import antenv
    mod = types.ModuleType("antenv.axon_hooks")
    mod.get_axon_ntff_profile_hook = lambda: _hook
    mod.set_axon_ntff_profile_hook = lambda h: None
    sys.modules["antenv.axon_hooks"] = mod
    antenv.axon_hooks = mod


def kernel(hidden_states, W):
    from concourse.bass_utils import run_bass_kernel_spmd

    x = np.ascontiguousarray(hidden_states, dtype=np.float32).reshape(N_TOK, HID)
    W = np.ascontiguousarray(W, dtype=np.float32)

    import ml_dtypes
    bf16 = ml_dtypes.bfloat16

    # core c, SBUF column j = i*TILE + p holds local token p*NTILES + i,
    # so staged outputs land contiguously as DRAM row p*NTILES+i.
    xr = x.reshape(N_CORES, TILE, NTILES, HID)
    xts = np.ascontiguousarray(xr.transpose(0, 3, 2, 1)).reshape(
        N_CORES, HID, TPC)
    xh = xts.astype(bf16)
    xl = (xts - xh.astype(np.float32)).astype(bf16)
    wT = np.ascontiguousarray(W.T)
    wh = wT.astype(bf16)
    wl = (wT - wh.astype(np.float32)).astype(bf16)
    wt = np.concatenate([wh, wl], axis=1)

    consts = np.zeros((2, GROUP * NE), dtype=np.uint32)
    consts[0] = np.tile(np.arange(NE, dtype=np.uint32), GROUP)
    consts[1] = 0xFFFFFFC0

    nc = _get_nc()
    in_maps = [{"xh": xh[c], "xl": xl[c], "wt": wt, "consts": consts}
               for c in range(N_CORES)]
    trace = bool(int(os.environ.get("KERNEL_TRACE", "0")))
    if trace:
        _ensure_axon_ntff_hook()
    res = run_bass_kernel_spmd(nc, in_maps, core_ids=list(range(N_CORES)),
                               trace=trace)
    if trace:
        _CACHE["exec_time_ns"] = res.exec_time_ns
        _CACHE["trace"] = res.instructions_and_trace

    vals = np.concatenate(
        [res.results[c]["vals"].reshape(TPC, 8) for c in range(N_CORES)])
    pi_parts = np.stack([res.results[c]["pi"] for c in range(N_CORES)])

    # expert index is stuffed in the low 6 bits of each f32 score
    idx8 = (vals.view(np.uint32) & np.uint32(63)).astype(np.int32)
    v6 = vals[:, :TOPK]
    topk_idx = idx8[:, :TOPK]
    topk_w = (v6 / (v6.sum(-1, keepdims=True) + 1e-20)).astype(np.float32)

    # host repair of ambiguous near-tie tokens (ACT exp is approximate)
    gaps_close = (vals[:, :TOPK] - vals[:, 1:TOPK + 1]) <= (
        TIE_REL_THRESH * np.abs(vals[:, :TOPK]))
    suspect = gaps_close.any(axis=1)
    if suspect.any():
        xs = x[suspect]
        lg = xs @ W.T
        lg -= lg.max(-1, keepdims=True)
        e = np.exp(lg)
        sc = (e / e.sum(-1, keepdims=True)).astype(np.float32)
        order = np.argsort(-sc, axis=-1, kind="stable")[:, :TOPK]
        tw = np.take_along_axis(sc, order, -1)
        tw = (tw / (tw.sum(-1, keepdims=True) + 1e-20)).astype(np.float32)
        topk_idx[suspect] = order.astype(np.int32)
        topk_w[suspect] = tw

    # aux loss: counts from the final indices, Pi from device partials
    pi_sum = np.zeros(NE, dtype=np.float64)
    for m in range(GROUP):
        pi_sum += pi_parts[:, m, m * NE:(m + 1) * NE].sum(axis=0)
    Pi = (pi_sum / N_TOK).astype(np.float32)
    counts = np.bincount(topk_idx.ravel(), minlength=NE).astype(np.float32)
    ce = counts / np.float32(topk_idx.size)
    fi = ce * np.float32(NE)
    aux_loss = np.float32((Pi * fi).sum() * ALPHA)

    return topk_idx, topk_w, aux_loss


# revision 26
# speedup vs baseline: 1.0992x; 1.0992x over previous
"""MoE gate (64 experts, top-6) on 8 Trainium2 NeuronCores, data-parallel.

Full inputs in, full outputs out. Token dim (N=262144) is sharded across the
8 cores; the tiny (64,128) gate weight is replicated. Per core the Bass/Tile
kernel computes, for each 128-token tile:
  PE   : logits = x_tile @ W.T            (fp32 matmul, x-slice stationary)
  ACT  : scores = exp(logits), Z = row-sum (fused accum_out)
  DVE  : top-8 score values + indices (max / max_index), 1/Z (batched recip)
  PE   : Pi-partial += rZ^T @ scores      (float32r stats matmul into PSUM)
Top-8 values/indices are staged in SBUF with a host-chosen token permutation
so every DMA is large and contiguous. The host finishes the job: normalizes
top-6 weights, bincounts expert usage, averages the Pi partials and forms the
aux loss. A small host-side repair pass recomputes tokens whose top-8 scores
have near-tie adjacent gaps (the ACT exp is a ~1e-5 approximation, so ties
within that noise could otherwise be ordered differently than the f32
reference).
"""

import os

import numpy as np

N_CORES = 8
HID = 128
NE = 64  # experts
TOPK = 6
ALPHA = 1e-3
N_TOK = 64 * 4096  # 262144 tokens total
TPC = N_TOK // N_CORES  # 32768 tokens per core
TILE = 128  # tokens per compute tile
NTILES = TPC // TILE  # 256
GROUP = 8  # tiles per group (ACT/stats batching)
CHUNK_TOK = 8192  # tokens per input DMA chunk
NCHUNK = TPC // CHUNK_TOK  # 4
TILES_PER_CHUNK = CHUNK_TOK // TILE  # 64
GROUPS_PER_CHUNK = TILES_PER_CHUNK // GROUP  # 8
NGROUPS = NCHUNK * GROUPS_PER_CHUNK  # 32

# relative gap under which adjacent top-8 ranks are considered ambiguous and
# the token is recomputed exactly on the host
TIE_REL_THRESH = 3e-4

_CACHE = {}


def _build():
    import concourse.bacc as bacc
    import concourse.mybir as mybir
    import concourse.tile as tile

    nc = bacc.Bacc("TRN2", target_bir_lowering=False, debug=False,
                   num_devices=N_CORES)
    f32 = mybir.dt.float32
    bf16 = mybir.dt.bfloat16
    u32 = mybir.dt.uint32
    Exp = mybir.ActivationFunctionType.Exp

    # x and W are shipped as bf16 hi/lo pairs; logits are rebuilt as
    # xh*wh + xh*wl + xl*wh + xl*wl (error ~1e-5, fixed by host repair)
    xh = nc.dram_tensor("xh", [HID, TPC], bf16, kind="ExternalInput")
    xl = nc.dram_tensor("xl", [HID, TPC], bf16, kind="ExternalInput")
    wt = nc.dram_tensor("wt", [HID, 2 * NE], bf16, kind="ExternalInput")
    # row 0: iota64 tiled (index-stuff pattern), row 1: 0xFFFFFFC0 mask
    consts = nc.dram_tensor("consts", [2, GROUP * NE], u32, kind="ExternalInput")
    # vals DRAM layout [128, NTILES*8]: partition p, free i*8+k holds the
    # rank-k stuffed score of local token p*NTILES+i (host permutes inputs
    # to match; expert index lives in the low 6 mantissa bits).
    vals = nc.dram_tensor("vals", [TILE, NTILES * 8], f32, kind="ExternalOutput")
    pi = nc.dram_tensor("pi", [GROUP, GROUP * NE], f32, kind="ExternalOutput")

    with tile.TileContext(nc) as tc:
        with (
            tc.tile_pool(name="singles", bufs=1) as singles,
            tc.tile_pool(name="xchunks", bufs=2) as xpool,
            tc.tile_pool(name="scorep", bufs=4) as spool,
            tc.tile_pool(name="zp", bufs=4) as zpool,
            tc.tile_pool(name="stagep", bufs=2) as stpool,
            tc.tile_pool(name="psuml", bufs=3, space="PSUM") as lpool,
            tc.tile_pool(name="psumstat", bufs=1, space="PSUM") as statpool,
        ):
            w_sb = singles.tile([HID, 2 * NE], bf16)
            nc.sync.dma_start(out=w_sb, in_=wt[:, :])
            iota_sb = singles.tile([TILE, GROUP * NE], u32)
            nc.sync.dma_start(
                out=iota_sb,
                in_=consts[0:1, :].to_broadcast([TILE, GROUP * NE]))
            mask_sb = singles.tile([TILE, 1], u32)
            nc.sync.dma_start(
                out=mask_sb, in_=consts[1:2, 0:1].to_broadcast([TILE, 1]))
            stats = statpool.tile([GROUP, GROUP * NE], f32)
            pending_stats = []

            SUB = 4  # sub-DMAs per chunk: first group starts sooner
            for c in range(NCHUNK):
                xh_t = xpool.tile([HID, CHUNK_TOK], bf16)
                xl_t = xpool.tile([HID, CHUNK_TOK], bf16)
                for s in range(SUB):
                    a = s * (CHUNK_TOK // SUB)
                    b = (s + 1) * (CHUNK_TOK // SUB)
                    nc.sync.dma_start(
                        out=xh_t[:, a:b],
                        in_=xh[:, c * CHUNK_TOK + a:c * CHUNK_TOK + b])
                    nc.sync.dma_start(
                        out=xl_t[:, a:b],
                        in_=xl[:, c * CHUNK_TOK + a:c * CHUNK_TOK + b])
                stage_v = stpool.tile([TILE, TILES_PER_CHUNK * 8], f32)
                for g in range(GROUPS_PER_CHUNK):
                    gi = c * GROUPS_PER_CHUNK + g
                    if g % 2 == 0:
                        zpair = zpool.tile([TILE, 2 * GROUP], f32)
                        rzpair = zpool.tile([TILE, 2 * GROUP], bf16)
                    lg = lpool.tile([TILE, GROUP * NE], f32)
                    scores = spool.tile([TILE, GROUP * NE], f32)
                    stuffed = spool.tile([TILE, GROUP * NE], u32)
                    scores_bf = spool.tile([TILE, GROUP * NE], bf16)
                    zblk = zpair[:, (g % 2) * GROUP:(g % 2) * GROUP + GROUP]
                    rz = rzpair[:, (g % 2) * GROUP:(g % 2) * GROUP + GROUP]
                    for m in range(GROUP):
                        t = g * GROUP + m  # tile within chunk
                        xs = slice(t * TILE, (t + 1) * TILE)
                        out = lg[:, m * NE:(m + 1) * NE]
                        nc.tensor.matmul(out, xh_t[:, xs], w_sb[:, :NE],
                                         start=True, stop=False)
                        nc.tensor.matmul(out, xh_t[:, xs], w_sb[:, NE:],
                                         start=False, stop=False)
                        nc.tensor.matmul(out, xl_t[:, xs], w_sb[:, :NE],
                                         start=False, stop=True)
                    nc.scalar.activation(scores[:, :], lg[:, :], Exp)
                    # overwrite each score's low 6 mantissa bits with its
                    # expert index: order-preserving, host decodes bits&63
                    nc.vector.scalar_tensor_tensor(
                        stuffed[:, :],
                        scores[:, :].bitcast(u32),
                        mask_sb[:, :],
                        iota_sb[:, :],
                        op0=mybir.AluOpType.bitwise_and,
                        op1=mybir.AluOpType.bitwise_or,
                    )
                    stuffed_f = stuffed[:, :].bitcast(f32)
                    with nc.allow_low_precision(
                            reason="Z/rz feed Pi stats only; bf16 noise "
                                   "averages out over 262k tokens"):
                        nc.scalar.copy(scores_bf[:, :], scores[:, :])
                        nc.vector.tensor_reduce(
                            zblk,
                            stuffed_f.rearrange("p (g e) -> p g e", e=NE),
                            axis=mybir.AxisListType.X,
                            op=mybir.AluOpType.add,
                        )
                        pending_stats.append((rz, scores_bf, gi))
                        if g % 2 == 1:
                            nc.vector.reciprocal(rzpair[:, :], zpair[:, :])
                            for prz, pbf, pgi in pending_stats:
                                nc.tensor.matmul(
                                    stats[:, :],
                                    prz,
                                    pbf[:, :],
                                    start=(pgi == 0),
                                    stop=(pgi == NGROUPS - 1),
                                )
                            pending_stats.clear()
                    for m in range(GROUP):
                        t = g * GROUP + m
                        nc.vector.max(
                            stage_v[:, t * 8:t * 8 + 8],
                            stuffed[:, m * NE:(m + 1) * NE].bitcast(f32),
                        )
                s0 = c * TILES_PER_CHUNK * 8
                s1 = (c + 1) * TILES_PER_CHUNK * 8
                nc.scalar.dma_start(out=vals[:, s0:s1], in_=stage_v)

            stats_sb = singles.tile([GROUP, GROUP * NE], f32)
            nc.vector.tensor_copy(stats_sb, stats)
            nc.scalar.dma_start(out=pi[:, :], in_=stats_sb)

    nc.compile()
    return nc


def _get_nc():
    if "nc" not in _CACHE:
        _CACHE["nc"] = _build()
    return _CACHE["nc"]


def _ensure_axon_ntff_hook():
    """Provide antenv.axon_hooks if the image lacks it (NTFF tracing)."""
    import contextlib
    import ctypes
    import sys
    import types

    try:
        from antenv.axon_hooks import get_axon_ntff_profile_hook  # noqa: F401
        return
    except ImportError:
        pass

    so_path = "/opt/axon/libaxon_pjrt.so"
    if not os.path.exists(so_path):
        return
    lib = ctypes.CDLL(so_path)
    if not hasattr(lib, "axon_start_nrt_profile"):
        return
    lib.axon_start_nrt_profile.argtypes = [
        ctypes.POINTER(ctypes.c_int64), ctypes.c_size_t]
    lib.axon_start_nrt_profile.restype = ctypes.c_int64
    lib.axon_stop_nrt_profile.argtypes = [ctypes.c_char_p]
    lib.axon_stop_nrt_profile.restype = ctypes.c_int64

    @contextlib.contextmanager
    def _hook(output_dir, device_ids):
        import jax
        jax.devices()
        if device_ids:
            ids = (ctypes.c_int64 * len(device_ids))(*device_ids)
            rc = lib.axon_start_nrt_profile(ids, len(device_ids))
        else:
            rc = lib.axon_start_nrt_profile(None, 0)
        if rc != 0:
            raise RuntimeError(f"axon_start_nrt_profile rc={rc}")
        try:
            yield
        finally:
            n = lib.axon_stop_nrt_profile(str(output_dir).encode())
            print(f"profile: {n} file(s) written to {output_dir}",
                  file=sys.stderr)

    import antenv
    mod = types.ModuleType("antenv.axon_hooks")
    mod.get_axon_ntff_profile_hook = lambda: _hook
    mod.set_axon_ntff_profile_hook = lambda h: None
    sys.modules["antenv.axon_hooks"] = mod
    antenv.axon_hooks = mod


def kernel(hidden_states, W):
    from concourse.bass_utils import run_bass_kernel_spmd

    x = np.ascontiguousarray(hidden_states, dtype=np.float32).reshape(N_TOK, HID)
    W = np.ascontiguousarray(W, dtype=np.float32)

    import ml_dtypes
    bf16 = ml_dtypes.bfloat16

    # core c, SBUF column j = i*TILE + p holds local token p*NTILES + i,
    # so staged outputs land contiguously as DRAM row p*NTILES+i.
    xr = x.reshape(N_CORES, TILE, NTILES, HID)
    xts = np.ascontiguousarray(xr.transpose(0, 3, 2, 1)).reshape(
        N_CORES, HID, TPC)
    xh = xts.astype(bf16)
    xl = (xts - xh.astype(np.float32)).astype(bf16)
    wT = np.ascontiguousarray(W.T)
    wh = wT.astype(bf16)
    wl = (wT - wh.astype(np.float32)).astype(bf16)
    wt = np.concatenate([wh, wl], axis=1)

    consts = np.zeros((2, GROUP * NE), dtype=np.uint32)
    consts[0] = np.tile(np.arange(NE, dtype=np.uint32), GROUP)
    consts[1] = 0xFFFFFFC0

    nc = _get_nc()
    in_maps = [{"xh": xh[c], "xl": xl[c], "wt": wt, "consts": consts}
               for c in range(N_CORES)]
    trace = bool(int(os.environ.get("KERNEL_TRACE", "0")))
    if trace:
        _ensure_axon_ntff_hook()
    res = run_bass_kernel_spmd(nc, in_maps, core_ids=list(range(N_CORES)),
                               trace=trace)
    if trace:
        _CACHE["exec_time_ns"] = res.exec_time_ns
        _CACHE["trace"] = res.instructions_and_trace

    vals = np.concatenate(
        [res.results[c]["vals"].reshape(TPC, 8) for c in range(N_CORES)])
    pi_parts = np.stack([res.results[c]["pi"] for c in range(N_CORES)])

    # expert index is stuffed in the low 6 bits of each f32 score
    idx8 = (vals.view(np.uint32) & np.uint32(63)).astype(np.int32)
    v6 = vals[:, :TOPK]
    topk_idx = idx8[:, :TOPK]
    topk_w = (v6 / (v6.sum(-1, keepdims=True) + 1e-20)).astype(np.float32)

    # host repair of ambiguous near-tie tokens (ACT exp is approximate)
    gaps_close = (vals[:, :TOPK] - vals[:, 1:TOPK + 1]) <= (
        TIE_REL_THRESH * np.abs(vals[:, :TOPK]))
    suspect = gaps_close.any(axis=1)
    if suspect.any():
        xs = x[suspect]
        lg = xs @ W.T
        lg -= lg.max(-1, keepdims=True)
        e = np.exp(lg)
        sc = (e / e.sum(-1, keepdims=True)).astype(np.float32)
        order = np.argsort(-sc, axis=-1, kind="stable")[:, :TOPK]
        tw = np.take_along_axis(sc, order, -1)
        tw = (tw / (tw.sum(-1, keepdims=True) + 1e-20)).astype(np.float32)
        topk_idx[suspect] = order.astype(np.int32)
        topk_w[suspect] = tw

    # aux loss: counts from the final indices, Pi from device partials
    pi_sum = np.zeros(NE, dtype=np.float64)
    for m in range(GROUP):
        pi_sum += pi_parts[:, m, m * NE:(m + 1) * NE].sum(axis=0)
    Pi = (pi_sum / N_TOK).astype(np.float32)
    counts = np.bincount(topk_idx.ravel(), minlength=NE).astype(np.float32)
    ce = counts / np.float32(topk_idx.size)
    fi = ce * np.float32(NE)
    aux_loss = np.float32((Pi * fi).sum() * ALPHA)

    return topk_idx, topk_w, aux_loss
